# revision 46
# baseline (speedup 1.0000x reference)
"""Trainium2 Bass kernel for nn_CrossAttention (dense_transformer).

Reference computation (per batch b, per stream s in {1,2}):
    q_s   = heads(x_s)                      # [H, N, D] slices of x_s
    kv_s  = x_s @ Wkv_s -> k_s, v_s         # [N, C] each
    gate_s= sigmoid(relu(x_s @ w1 + b1) @ w2 + b2)
    ctx_s = softmax_d( scale * k_s^T @ (v_s * gate_s) )   # [H, D, D], softmax over d
    o_1   = q_1 @ ctx_2 ; o_2 = q_2 @ ctx_1  (cross)

Sharding: 8 cores = (stream s, batch b) pairs.  Core (s, b) projects
x_s[b] (kv + gate + ctx_s[b]) and then computes the OTHER stream's
output o_{1-s}[b] = q_{1-s}[b] @ softmax(ctx_s[b]).  No cross-core
communication; host concatenates outputs.

Fully-fused single streaming pass (no DRAM spills).  kv path and
phase B in bf16 (host-converted); gate MLP in fp8 e4m3 with DoubleRow
matmuls (weights host-scaled by 16 to stay in the fp8 normal range,
compensated in the activation scale); PSUM-resident ctx accumulation
with (h, h+8) head pairs packed into distinct PE column groups;
output written bf16 and upcast on host.
"""

import numpy as np
from contextlib import ExitStack

N = 4096
C = 1024
H = 16
D = 64
SCALE = D ** (-0.5)
R = 512              # rows per A-phase block
NBLK = N // R        # 8 blocks
KCH = C // 128       # 8 contraction chunks

_CACHE = {}


def _build_program(with_bias):
    """Build the SPMD Bass program (same for all 8 cores)."""
    import concourse.bass as bass
    import concourse.bacc as bacc
    import concourse.tile as tile
    import concourse.mybir as mybir

    F32 = mybir.dt.float32
    BF16 = mybir.dt.bfloat16
    FP8 = mybir.dt.float8e4
    DR = mybir.MatmulPerfMode.DoubleRow
    AF = mybir.ActivationFunctionType
    WSCALE = 16.0  # fp8 gate weights are scaled by 16 on host (avoids subnormals)

    nc = bacc.Bacc("TRN2", target_bir_lowering=False, debug=False, num_devices=8)

    xp = nc.dram_tensor("xp", [N, C], BF16, kind="ExternalInput").ap()
    xq = nc.dram_tensor("xq", [N, C], BF16, kind="ExternalInput").ap()
    wkv = nc.dram_tensor("wkv", [C, 2 * C], BF16, kind="ExternalInput").ap()
    w1 = nc.dram_tensor("w1", [C, C], FP8, kind="ExternalInput").ap()
    b1 = nc.dram_tensor("b1", [C], F32, kind="ExternalInput").ap()
    w2 = nc.dram_tensor("w2", [C, C], FP8, kind="ExternalInput").ap()
    b2 = nc.dram_tensor("b2", [C], BF16, kind="ExternalInput").ap()
    identb = nc.dram_tensor("identb", [128, 128], BF16, kind="ExternalInput").ap()
    # identity replicated on both partition halves: ident2[p, c] = (p % 64 == c)
    ident2 = nc.dram_tensor("ident2", [128, 64], F32, kind="ExternalInput").ap()
    o = nc.dram_tensor("o", [N, C], BF16, kind="ExternalOutput").ap()

    with tile.TileContext(nc) as tc, ExitStack() as ctx:
        # ---------- pools ----------
        cpool = ctx.enter_context(tc.tile_pool(name="consts", bufs=1))
        wpool = ctx.enter_context(tc.tile_pool(name="weights", bufs=1))
        ctxp_pool = ctx.enter_context(tc.tile_pool(name="ctxps", bufs=1, space="PSUM"))
        mmps_pool = ctx.enter_context(tc.tile_pool(name="mmps", bufs=6, space="PSUM"))
        spool = ctx.enter_context(tc.tile_pool(name="spairs", bufs=1))
        xpin_pool = ctx.enter_context(tc.tile_pool(name="xpin", bufs=2))
        xqin_pool = ctx.enter_context(tc.tile_pool(name="xqin", bufs=3))
        xpT_pool = ctx.enter_context(tc.tile_pool(name="xpT", bufs=1))
        xpT8_pool = ctx.enter_context(tc.tile_pool(name="xpT8", bufs=1))
        hT_pool = ctx.enter_context(tc.tile_pool(name="hT", bufs=1))
        g_pool = ctx.enter_context(tc.tile_pool(name="g", bufs=4))
        k_pool = ctx.enter_context(tc.tile_pool(name="k", bufs=2))
        vg_pool = ctx.enter_context(tc.tile_pool(name="vg", bufs=2))
        xqT_pool = ctx.enter_context(tc.tile_pool(name="xqT", bufs=6))
        oout_pool = ctx.enter_context(tc.tile_pool(name="oout", bufs=6))

        def mmps(shape, dtype):
            return mmps_pool.tile(shape, dtype, name="mmps", tag="mmps")

        # ---------- DMA priority order ----------
        identb_sb = cpool.tile([128, 128], BF16, name="identb_sb")
        nc.sync.dma_start(identb_sb, identb)
        ident2_sb = cpool.tile([128, 64], F32, name="ident2_sb")
        nc.sync.dma_start(ident2_sb, ident2)
        b1_sb = cpool.tile([128, 8], F32, name="b1_sb")  # b1_sb[p, m] = b1[m*128+p]
        nc.sync.dma_start(b1_sb, b1.rearrange("(m p) -> p m", p=128))

        def x_dma(pool, src, blk, tag):
            t = pool.tile([128, 4, C], BF16, name=tag, tag=tag)
            nc.sync.dma_start(
                t, src[blk * R:(blk + 1) * R, :].rearrange("(c p) m -> p c m", p=128)
            )
            return t

        # xp block 0 and w1 are on the critical path: first, finely split
        def x_dma_half(pool, src, blk, hf, tag):
            t = pool.tile([128, 2, C], BF16, name=tag, tag=tag)
            nc.sync.dma_start(
                t,
                src[blk * R + hf * 256:blk * R + (hf + 1) * 256, :].rearrange(
                    "(c p) m -> p c m", p=128
                ),
            )
            return t

        xp0_halves = [x_dma_half(xpin_pool, xp, 0, hf, "xpin") for hf in range(2)]
        xp_tiles = {}
        w1_sb = wpool.tile([128, 8, C], FP8, name="w1_sb")  # [p, k, m]
        nc.sync.dma_start(w1_sb, w1.rearrange("(k p) m -> p k m", p=128))
        xq_tiles = {0: x_dma(xqin_pool, xq, 0, "xqin"),
                    1: x_dma(xqin_pool, xq, 1, "xqin")}
        w2_sb = wpool.tile([128, 8, C], FP8, name="w2_sb")
        nc.sync.dma_start(w2_sb, w2.rearrange("(k p) m -> p k m", p=128))
        wkv_sb = wpool.tile([128, 8, 2 * C], BF16, name="wkv_sb")
        nc.sync.dma_start(wkv_sb, wkv.rearrange("(k p) m -> p k m", p=128))
        if with_bias:
            ones_b = cpool.tile([1, 128], BF16, name="ones_b")
            nc.vector.memset(ones_b, 1.0)
            b2_r = cpool.tile([1, C], BF16, name="b2_r")
            nc.sync.dma_start(b2_r, b2.rearrange("(one f) -> one f", one=1))

        # ctx accumulators, ctxT layout [e, d] per head at cols (h%8)*64.
        # Bank A: heads 0-7 on partitions 0-63 (PE col groups 0-1).
        # Bank B: heads 8-15 on partitions 64-127 (PE col groups 2-3).
        # The (h, h+8) matmul pairs run concurrently on the PE.
        ctx_psA = ctxp_pool.tile([128, 512], F32, name="ctx_psA")
        ctx_psB = ctxp_pool.tile([128, 512], F32, name="ctx_psB")
        spairs = [spool.tile([128, 128], BF16, name=f"spair{j}") for j in range(8)]
        for j in range(8):  # pre-zero; softmax writes only the diagonal blocks
            nc.vector.memset(spairs[j], 0.0)

        xqT_tiles = {}

        def emit_trans_fn(chunk, out_pool, tag, fp8_pool=None):
            """chunk(c4) -> [128, C] bf16 slice; -> transposed [128, 8, 512].
            With fp8_pool, also emits an fp8 copy of the transposed tile."""
            xT = out_pool.tile([128, 8, R], BF16, name=tag, tag=tag)
            xT8 = (fp8_pool.tile([128, 8, R], FP8, name=tag + "8", tag=tag + "8")
                   if fp8_pool is not None else None)
            for j in range(8):
                trp = mmps([128, R], BF16)
                for c4 in range(4):
                    nc.tensor.transpose(
                        trp[:, c4 * 128:(c4 + 1) * 128],
                        chunk(c4)[:, j * 128:(j + 1) * 128],
                        identb_sb,
                    )
                if j % 2 == 0:
                    nc.vector.tensor_copy(xT[:, j, :], trp)
                    if xT8 is not None:
                        nc.scalar.copy(xT8[:, j, :], trp)
                else:
                    nc.scalar.copy(xT[:, j, :], trp)
                    if xT8 is not None:
                        nc.vector.tensor_copy(xT8[:, j, :], trp)
            return (xT, xT8) if fp8_pool is not None else xT

        def emit_trans(xin, out_pool, tag):
            return emit_trans_fn(lambda c4: xin[:, c4, :], out_pool, tag)

        def emit_xq_trans(blk):
            xqT_tiles[blk] = emit_trans(xq_tiles.pop(blk), xqT_pool, "xqT")

        # =========================================================
        # Phase A: per 512-row block: transpose -> gate1 -> gate2 ->
        #          kv -> ctx accumulation (PSUM)
        # =========================================================
        for blk in range(NBLK):
            # prefetch next xp block / upcoming xq blocks
            if blk + 1 < NBLK:
                xp_tiles[blk + 1] = x_dma(xpin_pool, xp, blk + 1, "xpin")
            if 1 <= blk <= 6:  # xq blocks 2..7 DMA'd one A-block early
                xq_tiles[blk + 1] = x_dma(xqin_pool, xq, blk + 1, "xqin")

            if blk == 0:
                xpT, xpT8 = emit_trans_fn(
                    lambda c4: xp0_halves[c4 // 2][:, c4 % 2, :], xpT_pool, "xpT",
                    fp8_pool=xpT8_pool,
                )
            else:
                xpT, xpT8 = xpT_next  # transposed during the previous block

            # ---- gate1 (fp8 DoubleRow): hT = relu(b1 + x @ (16*w1) / 16) ----
            hT = hT_pool.tile([128, 8, R], FP8, name="hT", tag="hT")
            for m in range(8):
                ps = mmps([128, R], F32)
                for kk in range(4):
                    nc.tensor.matmul(
                        ps,
                        w1_sb[:, 2 * kk:2 * kk + 2, m * 128:(m + 1) * 128],
                        xpT8[:, 2 * kk:2 * kk + 2, :],
                        start=(kk == 0),
                        stop=(kk == 3),
                        perf_mode=DR,
                    )
                nc.scalar.activation(
                    hT[:, m, :], ps, AF.Relu, bias=b1_sb[:, m:m + 1],
                    scale=1.0 / WSCALE,
                )

            if blk == 0:
                # xq transposes here cover the w2/wkv weight-load latency
                emit_xq_trans(0)
                emit_xq_trans(1)

            # ---- gate2: g[n, :] = sigmoid(sum_k hT[k]^T w2[k] + b2) ----
            gts = []
            for c4 in range(4):
                gt = g_pool.tile([128, C], BF16, name="gt", tag="gt")
                for t in range(2):
                    ps = mmps([128, 512], F32)
                    for kk in range(4):
                        nc.tensor.matmul(
                            ps,
                            hT[:, 2 * kk:2 * kk + 2, c4 * 128:(c4 + 1) * 128],
                            w2_sb[:, 2 * kk:2 * kk + 2, t * 512:(t + 1) * 512],
                            start=(kk == 0),
                            stop=(kk == 3 and not with_bias),
                            perf_mode=DR,
                        )
                    if with_bias:
                        # b2 is host-scaled by WSCALE to survive the 1/WSCALE
                        nc.tensor.matmul(
                            ps,
                            ones_b,
                            b2_r[:, t * 512:(t + 1) * 512],
                            start=False,
                            stop=True,
                        )
                    nc.scalar.activation(
                        gt[:, t * 512:(t + 1) * 512], ps, AF.Sigmoid,
                        scale=1.0 / WSCALE,
                    )
                gts.append(gt)

            # ---- kv projection + ctx accumulation, per 128-row chunk ----
            def emit_kv(c4):
                k_bf = k_pool.tile([128, C], BF16, name="k_bf", tag="k_bf")
                vg = vg_pool.tile([128, C], BF16, name="vg", tag="vg")
                for t in range(4):
                    ps = mmps([128, 512], F32)
                    for k in range(8):
                        nc.tensor.matmul(
                            ps,
                            xpT[:, k, c4 * 128:(c4 + 1) * 128],
                            wkv_sb[:, k, t * 512:(t + 1) * 512],
                            start=(k == 0),
                            stop=(k == 7),
                        )
                    if t < 2:
                        nc.scalar.copy(k_bf[:, t * 512:(t + 1) * 512], ps)
                    else:
                        nc.vector.tensor_mul(
                            vg[:, (t - 2) * 512:(t - 1) * 512],
                            ps,
                            gts[c4][:, (t - 2) * 512:(t - 1) * 512],
                        )
                return k_bf, vg

            def emit_ctx(c4, kv_tiles):
                k_bf, vg = kv_tiles
                first = (blk == 0 and c4 == 0)
                last = (blk == NBLK - 1 and c4 == 3)
                for hc in range(8):
                    for hp in range(2):
                        h = hp * 8 + hc
                        dst = ctx_psA if hp == 0 else ctx_psB
                        # start=True clears has_written for the whole bank ->
                        # exactly one clearing matmul per bank.
                        nc.tensor.matmul(
                            dst[hp * 64:(hp + 1) * 64, hc * 64:(hc + 1) * 64],
                            vg[:, h * D:(h + 1) * D],
                            k_bf[:, h * D:(h + 1) * D],
                            start=(first and hc == 0),
                            stop=last,
                            skip_group_check=True,
                            tile_position=(0, hp * 64),
                        )

            kvt = {}
            kvt[0] = emit_kv(0)
            kvt[1] = emit_kv(1)
            emit_ctx(0, kvt[0])
            kvt[2] = emit_kv(2)
            emit_ctx(1, kvt[1])
            kvt[3] = emit_kv(3)
            if blk + 1 < NBLK:
                # next block's transposes here so their PSUM->SBUF drains
                # complete under the remaining ctx work (no gate1 stall)
                xpT_next = emit_trans_fn(
                    lambda c4, t=xp_tiles.pop(blk + 1): t[:, c4, :], xpT_pool,
                    "xpT", fp8_pool=xpT8_pool,
                )
            emit_ctx(2, kvt[2])
            emit_ctx(3, kvt[3])

            if 2 <= blk <= 4:  # spread xq transposes through phase A
                emit_xq_trans(blk)

        # =========================================================
        # Softmax over d (free dim of ctxT) + build block-diag S pairs
        # st layout: heads 0-7 on partitions 0-63, heads 8-15 on 64-127.
        # =========================================================
        # The softmax chain is emitted before trans(5) so DVE prioritizes it;
        # trans(5) keeps the PE busy during the chain, draining via ScalarE.
        with ExitStack() as sm:
            smp = sm.enter_context(tc.tile_pool(name="smpool", bufs=1))
            maxs = smp.tile([128, 8], F32, name="maxs")
            cmx = smp.tile([128, 512], F32, name="cmx")
            sums = smp.tile([128, 8], F32, name="sums")
            et = mmps([128, 512], F32)
            halves = [(ctx_psA, slice(0, 64)), (ctx_psB, slice(64, 128))]
            for cps, sl in halves:
                nc.vector.tensor_reduce(
                    maxs[sl, :],
                    cps[sl, :].rearrange("p (b d) -> p b d", b=8),
                    axis=mybir.AxisListType.X,
                    op=mybir.AluOpType.max,
                )
                nc.vector.tensor_sub(
                    cmx[sl, :].rearrange("p (h d) -> p h d", h=8),
                    cps[sl, :].rearrange("p (h d) -> p h d", h=8),
                    maxs[sl, :].unsqueeze(-1).broadcast_to([64, 8, 64]),
                )
                nc.scalar.activation(
                    et[sl, :], cmx[sl, :], AF.Exp, scale=float(SCALE)
                )
            nc.vector.tensor_reduce(
                sums,
                et.rearrange("p (b d) -> p b d", b=8),
                axis=mybir.AxisListType.X,
                op=mybir.AluOpType.add,
            )
            recs = smp.tile([128, 8], F32, name="recs")
            nc.vector.reciprocal(recs, sums)
            st = smp.tile([128, 512], F32, name="st")
            nc.vector.tensor_mul(
                st.rearrange("p (h d) -> p h d", h=8),
                et.rearrange("p (h d) -> p h d", h=8),
                recs.unsqueeze(-1).broadcast_to([128, 8, 64]),
            )
            # PE cover for the chain above; drains on ScalarE to keep DVE free
            xqT5 = xqT_pool.tile([128, 8, R], BF16, name="xqT", tag="xqT")
            xq5 = xq_tiles.pop(5)
            for j in range(8):
                trp = mmps([128, R], BF16)
                for c4 in range(4):
                    nc.tensor.transpose(
                        trp[:, c4 * 128:(c4 + 1) * 128],
                        xq5[:, c4, j * 128:(j + 1) * 128],
                        identb_sb,
                    )
                nc.scalar.copy(xqT5[:, j, :], trp)
            xqT_tiles[5] = xqT5
            # Transposing the pair [ctxT_2j | ctxT_2j+1] ([64, 128]) gives
            # [S_2j stacked above S_2j+1] ([128, 64]); scatter to block-diag.
            for j in range(8):
                sl = slice(0, 64) if j < 4 else slice(64, 128)
                col = (2 * j) * 64 % 512
                tp = mmps([128, 64], F32)
                nc.tensor.transpose(
                    tp, st[sl, col:col + 128], ident2_sb[sl, :]
                )
                if j % 2 == 0:
                    nc.vector.tensor_copy(spairs[j][0:64, 0:64], tp[0:64, :])
                    nc.scalar.copy(spairs[j][64:128, 64:128], tp[64:128, :])
                else:
                    nc.scalar.copy(spairs[j][0:64, 0:64], tp[0:64, :])
                    nc.vector.tensor_copy(spairs[j][64:128, 64:128], tp[64:128, :])

        # =========================================================
        # Phase B: o[nchunk, j*128:(j+1)*128] = (xqT_j_chunk).T @ spair_j
        # =========================================================
        def emit_b(blk):
            xqT = xqT_tiles.pop(blk)
            # chunks per output DMA: 1 for the last block (fast final flush)
            grp = 1 if blk == NBLK - 1 else 2
            for ch in range(4 // grp):
                oout = oout_pool.tile([128, 2, C], BF16, name="oo", tag="oo")
                for cc in range(grp):
                    c4 = ch * grp + cc
                    for half in range(2):
                        ps = mmps([128, 512], F32)
                        for jj in range(4):
                            j = half * 4 + jj
                            nc.tensor.matmul(
                                ps[:, jj * 128:(jj + 1) * 128],
                                xqT[:, j, c4 * 128:(c4 + 1) * 128],
                                spairs[j],
                                start=True,
                                stop=True,
                                skip_group_check=True,
                            )
                        if half == 0:
                            nc.vector.tensor_copy(oout[:, cc, 0:512], ps)
                        else:
                            nc.scalar.copy(oout[:, cc, 512:1024], ps)
                r0 = blk * R + ch * 128 * grp
                nc.sync.dma_start(
                    o[r0:r0 + 128 * grp, :].rearrange("(c p) m -> p c m", p=128),
                    oout[:, 0:grp, :],
                )

        emit_b(0)
        emit_xq_trans(6)
        emit_b(1)
        emit_xq_trans(7)
        for blk in range(2, NBLK):
            emit_b(blk)

    nc.compile()
    return nc


def _get_program(with_bias=False):
    key = ("nc", bool(with_bias))
    if key not in _CACHE:
        _CACHE[key] = _build_program(with_bias)
    return _CACHE[key]


def make_in_maps(x1, x2, Wkv1, Wkv2, g1_w1, g1_b1, g1_w2, g1_b2,
                 g2_w1, g2_b1, g2_w2, g2_b2):
    """Core (s, b): cores 0-3 = (s=0, b), cores 4-7 = (s=1, b)."""
    import ml_dtypes
    BF = ml_dtypes.bfloat16
    identb = np.eye(128, dtype=BF)
    eye64 = np.eye(64, dtype=np.float32)
    ident2 = np.ascontiguousarray(np.concatenate([eye64, eye64], axis=0))

    def bf(a):
        return np.ascontiguousarray(np.asarray(a, np.float32).astype(BF))

    WSCALE = 16.0  # gate weights scaled into the fp8 normal range

    def f8(a):
        return np.ascontiguousarray(
            (np.asarray(a, np.float32) * WSCALE).astype(ml_dtypes.float8_e4m3fn)
        )

    x1b = [bf(x1[b]) for b in range(x1.shape[0])]
    x2b = [bf(x2[b]) for b in range(x2.shape[0])]
    Ws = [
        dict(wkv=bf(Wkv1), w1=f8(g1_w1), b1=np.asarray(g1_b1, np.float32),
             w2=f8(g1_w2), b2=bf(np.asarray(g1_b2, np.float32) * WSCALE)),
        dict(wkv=bf(Wkv2), w1=f8(g2_w1), b1=np.asarray(g2_b1, np.float32),
             w2=f8(g2_w2), b2=bf(np.asarray(g2_b2, np.float32) * WSCALE)),
    ]
    in_maps = []
    for core in range(8):
        s, b = core // 4, core % 4
        m = dict(Ws[s])
        m["xp"] = x1b[b] if s == 0 else x2b[b]
        m["xq"] = x2b[b] if s == 0 else x1b[b]
        m["identb"] = identb
        m["ident2"] = ident2
        in_maps.append(m)
    return in_maps


def kernel(x1, x2, Wkv1, Wkv2, g1_w1, g1_b1, g1_w2, g1_b2,
           g2_w1, g2_b1, g2_w2, g2_b2, _runner=None):
    """Full-input entry point.  Returns (o1, o2), each [4, 4096, 1024] f32."""
    from concourse.bass_utils import run_bass_kernel_spmd

    args = [np.asarray(a, dtype=np.float32) for a in
            (x1, x2, Wkv1, Wkv2, g1_w1, g1_b1, g1_w2, g1_b2,
             g2_w1, g2_b1, g2_w2, g2_b2)]
    with_bias = bool(np.any(args[7]) or np.any(args[11]))  # g1_b2, g2_b2
    nc = _get_program(with_bias)
    in_maps = make_in_maps(*args)
    if _runner is None:
        res = run_bass_kernel_spmd(nc, in_maps, core_ids=list(range(8)))
        results = res.results
    else:
        results = _runner(nc, in_maps)

    B = x1.shape[0]
    o1 = np.empty((B, N, C), dtype=np.float32)
    o2 = np.empty((B, N, C), dtype=np.float32)
    for core in range(8):
        s, b = core // 4, core % 4
        out = np.asarray(results[core]["o"], dtype=np.float32)
        if s == 0:
            o2[b] = out   # core projected x1 -> ctx1 -> o2 = q2 @ ctx1
        else:
            o1[b] = out
    return (o1, o2)


# revision 51
# speedup vs baseline: 1.0045x; 1.0045x over previous
"""Trainium2 Bass kernel for nn_CrossAttention (dense_transformer).

Reference computation (per batch b, per stream s in {1,2}):
    q_s   = heads(x_s)                      # [H, N, D] slices of x_s
    kv_s  = x_s @ Wkv_s -> k_s, v_s         # [N, C] each
    gate_s= sigmoid(relu(x_s @ w1 + b1) @ w2 + b2)
    ctx_s = softmax_d( scale * k_s^T @ (v_s * gate_s) )   # [H, D, D], softmax over d
    o_1   = q_1 @ ctx_2 ; o_2 = q_2 @ ctx_1  (cross)

Sharding: 8 cores = (stream s, batch b) pairs.  Core (s, b) projects
x_s[b] (kv + gate + ctx_s[b]) and then computes the OTHER stream's
output o_{1-s}[b] = q_{1-s}[b] @ softmax(ctx_s[b]).  No cross-core
communication; host concatenates outputs.

Fully-fused single streaming pass (no DRAM spills).  kv path and
phase B in bf16 (host-converted); gate MLP in fp8 e4m3 with DoubleRow
matmuls (weights host-scaled by 16 to stay in the fp8 normal range,
compensated in the activation scale); PSUM-resident ctx accumulation
with (h, h+8) head pairs packed into distinct PE column groups;
output written bf16 and upcast on host.
"""

import numpy as np
from contextlib import ExitStack

N = 4096
C = 1024
H = 16
D = 64
SCALE = D ** (-0.5)
R = 512              # rows per A-phase block
NBLK = N // R        # 8 blocks
KCH = C // 128       # 8 contraction chunks

_CACHE = {}


def _build_program(with_bias):
    """Build the SPMD Bass program (same for all 8 cores)."""
    import concourse.bass as bass
    import concourse.bacc as bacc
    import concourse.tile as tile
    import concourse.mybir as mybir

    F32 = mybir.dt.float32
    BF16 = mybir.dt.bfloat16
    FP8 = mybir.dt.float8e4
    DR = mybir.MatmulPerfMode.DoubleRow
    AF = mybir.ActivationFunctionType
    WSCALE = 16.0  # fp8 gate weights are scaled by 16 on host (avoids subnormals)

    nc = bacc.Bacc("TRN2", target_bir_lowering=False, debug=False, num_devices=8)

    xp = nc.dram_tensor("xp", [N, C], BF16, kind="ExternalInput").ap()
    xq = nc.dram_tensor("xq", [N, C], BF16, kind="ExternalInput").ap()
    wkv = nc.dram_tensor("wkv", [C, 2 * C], BF16, kind="ExternalInput").ap()
    w1 = nc.dram_tensor("w1", [C, C], FP8, kind="ExternalInput").ap()
    b1 = nc.dram_tensor("b1", [C], F32, kind="ExternalInput").ap()
    w2 = nc.dram_tensor("w2", [C, C], FP8, kind="ExternalInput").ap()
    b2 = nc.dram_tensor("b2", [C], BF16, kind="ExternalInput").ap()
    identb = nc.dram_tensor("identb", [128, 128], BF16, kind="ExternalInput").ap()
    # identity replicated on both partition halves: ident2[p, c] = (p % 64 == c)
    ident2 = nc.dram_tensor("ident2", [128, 64], F32, kind="ExternalInput").ap()
    o = nc.dram_tensor("o", [N, C], BF16, kind="ExternalOutput").ap()

    with tile.TileContext(nc) as tc, ExitStack() as ctx:
        # ---------- pools ----------
        cpool = ctx.enter_context(tc.tile_pool(name="consts", bufs=1))
        wpool = ctx.enter_context(tc.tile_pool(name="weights", bufs=1))
        ctxp_pool = ctx.enter_context(tc.tile_pool(name="ctxps", bufs=1, space="PSUM"))
        mmps_pool = ctx.enter_context(tc.tile_pool(name="mmps", bufs=6, space="PSUM"))
        spool = ctx.enter_context(tc.tile_pool(name="spairs", bufs=1))
        xpin_pool = ctx.enter_context(tc.tile_pool(name="xpin", bufs=2))
        xp0q_pool = ctx.enter_context(tc.tile_pool(name="xp0q", bufs=4))
        xqin_pool = ctx.enter_context(tc.tile_pool(name="xqin", bufs=3))
        xpT_pool = ctx.enter_context(tc.tile_pool(name="xpT", bufs=1))
        xpT8_pool = ctx.enter_context(tc.tile_pool(name="xpT8", bufs=1))
        hT_pool = ctx.enter_context(tc.tile_pool(name="hT", bufs=1))
        g_pool = ctx.enter_context(tc.tile_pool(name="g", bufs=4))
        k_pool = ctx.enter_context(tc.tile_pool(name="k", bufs=2))
        vg_pool = ctx.enter_context(tc.tile_pool(name="vg", bufs=2))
        xqT_pool = ctx.enter_context(tc.tile_pool(name="xqT", bufs=6))
        oout_pool = ctx.enter_context(tc.tile_pool(name="oout", bufs=6))

        def mmps(shape, dtype):
            return mmps_pool.tile(shape, dtype, name="mmps", tag="mmps")

        # ---------- DMA priority order ----------
        identb_sb = cpool.tile([128, 128], BF16, name="identb_sb")
        nc.sync.dma_start(identb_sb, identb)
        ident2_sb = cpool.tile([128, 64], F32, name="ident2_sb")
        nc.sync.dma_start(ident2_sb, ident2)
        b1_sb = cpool.tile([128, 8], F32, name="b1_sb")  # b1_sb[p, m] = b1[m*128+p]
        nc.sync.dma_start(b1_sb, b1.rearrange("(m p) -> p m", p=128))

        def x_dma(pool, src, blk, tag):
            t = pool.tile([128, 4, C], BF16, name=tag, tag=tag)
            nc.sync.dma_start(
                t, src[blk * R:(blk + 1) * R, :].rearrange("(c p) m -> p c m", p=128)
            )
            return t

        # startup critical path: xp0 (quartered, PE starts ASAP), then ALL
        # weights in need-order before any xq traffic shares the bandwidth
        def x_dma_q(pool, src, blk, c4, tag):
            t = pool.tile([128, 1, C], BF16, name=tag, tag=tag)
            r0 = blk * R + c4 * 128
            nc.sync.dma_start(
                t, src[r0:r0 + 128, :].rearrange("(c p) m -> p c m", p=128)
            )
            return t

        xp0_quarters = [x_dma_q(xp0q_pool, xp, 0, c4, "xp0q") for c4 in range(4)]
        xp_tiles = {}
        xq_tiles = {}
        w1_sb = wpool.tile([128, 8, C], FP8, name="w1_sb")  # [p, k, m]
        nc.sync.dma_start(w1_sb, w1.rearrange("(k p) m -> p k m", p=128))
        w2_sb = wpool.tile([128, 8, C], FP8, name="w2_sb")
        nc.sync.dma_start(w2_sb, w2.rearrange("(k p) m -> p k m", p=128))
        wkv_sb = wpool.tile([128, 8, 2 * C], BF16, name="wkv_sb")
        nc.sync.dma_start(wkv_sb, wkv.rearrange("(k p) m -> p k m", p=128))
        if with_bias:
            ones_b = cpool.tile([1, 128], BF16, name="ones_b")
            nc.vector.memset(ones_b, 1.0)
            b2_r = cpool.tile([1, C], BF16, name="b2_r")
            nc.sync.dma_start(b2_r, b2.rearrange("(one f) -> one f", one=1))

        # ctx accumulators, ctxT layout [e, d] per head at cols (h%8)*64.
        # Bank A: heads 0-7 on partitions 0-63 (PE col groups 0-1).
        # Bank B: heads 8-15 on partitions 64-127 (PE col groups 2-3).
        # The (h, h+8) matmul pairs run concurrently on the PE.
        ctx_psA = ctxp_pool.tile([128, 512], F32, name="ctx_psA")
        ctx_psB = ctxp_pool.tile([128, 512], F32, name="ctx_psB")
        spairs = [spool.tile([128, 128], BF16, name=f"spair{j}") for j in range(8)]
        for j in range(8):  # pre-zero; softmax writes only the diagonal blocks
            nc.vector.memset(spairs[j], 0.0)

        xqT_tiles = {}

        def emit_trans_fn(chunk, out_pool, tag, fp8_pool=None):
            """chunk(c4) -> [128, C] bf16 slice; -> transposed [128, 8, 512].
            With fp8_pool, also emits an fp8 copy of the transposed tile."""
            xT = out_pool.tile([128, 8, R], BF16, name=tag, tag=tag)
            xT8 = (fp8_pool.tile([128, 8, R], FP8, name=tag + "8", tag=tag + "8")
                   if fp8_pool is not None else None)
            for j in range(8):
                trp = mmps([128, R], BF16)
                for c4 in range(4):
                    nc.tensor.transpose(
                        trp[:, c4 * 128:(c4 + 1) * 128],
                        chunk(c4)[:, j * 128:(j + 1) * 128],
                        identb_sb,
                    )
                if j % 2 == 0:
                    nc.vector.tensor_copy(xT[:, j, :], trp)
                    if xT8 is not None:
                        nc.scalar.copy(xT8[:, j, :], trp)
                else:
                    nc.scalar.copy(xT[:, j, :], trp)
                    if xT8 is not None:
                        nc.vector.tensor_copy(xT8[:, j, :], trp)
            return (xT, xT8) if fp8_pool is not None else xT

        def emit_trans(xin, out_pool, tag):
            return emit_trans_fn(lambda c4: xin[:, c4, :], out_pool, tag)

        def emit_xq_trans(blk):
            xqT_tiles[blk] = emit_trans(xq_tiles.pop(blk), xqT_pool, "xqT")

        # =========================================================
        # Phase A: per 512-row block: transpose -> gate1 -> gate2 ->
        #          kv -> ctx accumulation (PSUM)
        # =========================================================
        for blk in range(NBLK):
            # prefetch next xp block / upcoming xq blocks
            if blk + 1 < NBLK:
                xp_tiles[blk + 1] = x_dma(xpin_pool, xp, blk + 1, "xpin")
            if blk == 0:  # xq DMAs start only after all weight DMAs
                xq_tiles[0] = x_dma(xqin_pool, xq, 0, "xqin")
                xq_tiles[1] = x_dma(xqin_pool, xq, 1, "xqin")
            elif blk <= 6:
                xq_tiles[blk + 1] = x_dma(xqin_pool, xq, blk + 1, "xqin")

            if blk == 0:
                xpT, xpT8 = emit_trans_fn(
                    lambda c4: xp0_quarters[c4][:, 0, :], xpT_pool, "xpT",
                    fp8_pool=xpT8_pool,
                )
            else:
                xpT, xpT8 = xpT_next  # transposed during the previous block

            # ---- gate1 (fp8 DoubleRow): hT = relu(b1 + x @ (16*w1) / 16) ----
            hT = hT_pool.tile([128, 8, R], FP8, name="hT", tag="hT")
            for m in range(8):
                ps = mmps([128, R], F32)
                for kk in range(4):
                    nc.tensor.matmul(
                        ps,
                        w1_sb[:, 2 * kk:2 * kk + 2, m * 128:(m + 1) * 128],
                        xpT8[:, 2 * kk:2 * kk + 2, :],
                        start=(kk == 0),
                        stop=(kk == 3),
                        perf_mode=DR,
                    )
                nc.scalar.activation(
                    hT[:, m, :], ps, AF.Relu, bias=b1_sb[:, m:m + 1],
                    scale=1.0 / WSCALE,
                )

            # ---- gate2: g[n, :] = sigmoid(sum_k hT[k]^T w2[k] + b2) ----
            gts = []
            for c4 in range(4):
                gt = g_pool.tile([128, C], BF16, name="gt", tag="gt")
                for t in range(2):
                    ps = mmps([128, 512], F32)
                    for kk in range(4):
                        nc.tensor.matmul(
                            ps,
                            hT[:, 2 * kk:2 * kk + 2, c4 * 128:(c4 + 1) * 128],
                            w2_sb[:, 2 * kk:2 * kk + 2, t * 512:(t + 1) * 512],
                            start=(kk == 0),
                            stop=(kk == 3 and not with_bias),
                            perf_mode=DR,
                        )
                    if with_bias:
                        # b2 is host-scaled by WSCALE to survive the 1/WSCALE
                        nc.tensor.matmul(
                            ps,
                            ones_b,
                            b2_r[:, t * 512:(t + 1) * 512],
                            start=False,
                            stop=True,
                        )
                    nc.scalar.activation(
                        gt[:, t * 512:(t + 1) * 512], ps, AF.Sigmoid,
                        scale=1.0 / WSCALE,
                    )
                gts.append(gt)

            # ---- kv projection + ctx accumulation, per 128-row chunk ----
            def emit_kv(c4):
                k_bf = k_pool.tile([128, C], BF16, name="k_bf", tag="k_bf")
                vg = vg_pool.tile([128, C], BF16, name="vg", tag="vg")
                for t in range(4):
                    ps = mmps([128, 512], F32)
                    for k in range(8):
                        nc.tensor.matmul(
                            ps,
                            xpT[:, k, c4 * 128:(c4 + 1) * 128],
                            wkv_sb[:, k, t * 512:(t + 1) * 512],
                            start=(k == 0),
                            stop=(k == 7),
                        )
                    if t < 2:
                        nc.scalar.copy(k_bf[:, t * 512:(t + 1) * 512], ps)
                    else:
                        nc.vector.tensor_mul(
                            vg[:, (t - 2) * 512:(t - 1) * 512],
                            ps,
                            gts[c4][:, (t - 2) * 512:(t - 1) * 512],
                        )
                return k_bf, vg

            def emit_ctx(c4, kv_tiles):
                k_bf, vg = kv_tiles
                first = (blk == 0 and c4 == 0)
                last = (blk == NBLK - 1 and c4 == 3)
                for hc in range(8):
                    for hp in range(2):
                        h = hp * 8 + hc
                        dst = ctx_psA if hp == 0 else ctx_psB
                        # start=True clears has_written for the whole bank ->
                        # exactly one clearing matmul per bank.
                        nc.tensor.matmul(
                            dst[hp * 64:(hp + 1) * 64, hc * 64:(hc + 1) * 64],
                            vg[:, h * D:(h + 1) * D],
                            k_bf[:, h * D:(h + 1) * D],
                            start=(first and hc == 0),
                            stop=last,
                            skip_group_check=True,
                            tile_position=(0, hp * 64),
                        )

            kvt = {}
            kvt[0] = emit_kv(0)
            kvt[1] = emit_kv(1)
            emit_ctx(0, kvt[0])
            kvt[2] = emit_kv(2)
            emit_ctx(1, kvt[1])
            kvt[3] = emit_kv(3)
            if blk + 1 < NBLK:
                # next block's transposes here so their PSUM->SBUF drains
                # complete under the remaining ctx work (no gate1 stall)
                xpT_next = emit_trans_fn(
                    lambda c4, t=xp_tiles.pop(blk + 1): t[:, c4, :], xpT_pool,
                    "xpT", fp8_pool=xpT8_pool,
                )
            emit_ctx(2, kvt[2])
            emit_ctx(3, kvt[3])

            if 1 <= blk <= 5:  # spread xq transposes through phase A
                emit_xq_trans(blk - 1)

        # =========================================================
        # Softmax over d (free dim of ctxT) + build block-diag S pairs
        # st layout: heads 0-7 on partitions 0-63, heads 8-15 on 64-127.
        # =========================================================
        # The softmax chain is emitted before trans(5) so DVE prioritizes it;
        # trans(5) keeps the PE busy during the chain, draining via ScalarE.
        with ExitStack() as sm:
            smp = sm.enter_context(tc.tile_pool(name="smpool", bufs=1))
            maxs = smp.tile([128, 8], F32, name="maxs")
            cmx = smp.tile([128, 512], F32, name="cmx")
            sums = smp.tile([128, 8], F32, name="sums")
            et = mmps([128, 512], F32)
            halves = [(ctx_psA, slice(0, 64)), (ctx_psB, slice(64, 128))]
            for cps, sl in halves:
                nc.vector.tensor_reduce(
                    maxs[sl, :],
                    cps[sl, :].rearrange("p (b d) -> p b d", b=8),
                    axis=mybir.AxisListType.X,
                    op=mybir.AluOpType.max,
                )
                nc.vector.tensor_sub(
                    cmx[sl, :].rearrange("p (h d) -> p h d", h=8),
                    cps[sl, :].rearrange("p (h d) -> p h d", h=8),
                    maxs[sl, :].unsqueeze(-1).broadcast_to([64, 8, 64]),
                )
                nc.scalar.activation(
                    et[sl, :], cmx[sl, :], AF.Exp, scale=float(SCALE)
                )
            nc.vector.tensor_reduce(
                sums,
                et.rearrange("p (b d) -> p b d", b=8),
                axis=mybir.AxisListType.X,
                op=mybir.AluOpType.add,
            )
            recs = smp.tile([128, 8], F32, name="recs")
            nc.vector.reciprocal(recs, sums)
            st = smp.tile([128, 512], F32, name="st")
            nc.vector.tensor_mul(
                st.rearrange("p (h d) -> p h d", h=8),
                et.rearrange("p (h d) -> p h d", h=8),
                recs.unsqueeze(-1).broadcast_to([128, 8, 64]),
            )
            # PE cover for the chain above; drains on ScalarE to keep DVE free
            xqT5 = xqT_pool.tile([128, 8, R], BF16, name="xqT", tag="xqT")
            xq5 = xq_tiles.pop(5)
            for j in range(8):
                trp = mmps([128, R], BF16)
                for c4 in range(4):
                    nc.tensor.transpose(
                        trp[:, c4 * 128:(c4 + 1) * 128],
                        xq5[:, c4, j * 128:(j + 1) * 128],
                        identb_sb,
                    )
                nc.scalar.copy(xqT5[:, j, :], trp)
            xqT_tiles[5] = xqT5
            # Transposing the pair [ctxT_2j | ctxT_2j+1] ([64, 128]) gives
            # [S_2j stacked above S_2j+1] ([128, 64]); scatter to block-diag.
            for j in range(8):
                sl = slice(0, 64) if j < 4 else slice(64, 128)
                col = (2 * j) * 64 % 512
                tp = mmps([128, 64], F32)
                nc.tensor.transpose(
                    tp, st[sl, col:col + 128], ident2_sb[sl, :]
                )
                if j % 2 == 0:
                    nc.vector.tensor_copy(spairs[j][0:64, 0:64], tp[0:64, :])
                    nc.scalar.copy(spairs[j][64:128, 64:128], tp[64:128, :])
                else:
                    nc.scalar.copy(spairs[j][0:64, 0:64], tp[0:64, :])
                    nc.vector.tensor_copy(spairs[j][64:128, 64:128], tp[64:128, :])

        # =========================================================
        # Phase B: o[nchunk, j*128:(j+1)*128] = (xqT_j_chunk).T @ spair_j
        # =========================================================
        def emit_b(blk):
            xqT = xqT_tiles.pop(blk)
            # chunks per output DMA: 1 for the last block (fast final flush)
            grp = 1 if blk == NBLK - 1 else 2
            for ch in range(4 // grp):
                oout = oout_pool.tile([128, 2, C], BF16, name="oo", tag="oo")
                for cc in range(grp):
                    c4 = ch * grp + cc
                    for half in range(2):
                        ps = mmps([128, 512], F32)
                        for jj in range(4):
                            j = half * 4 + jj
                            nc.tensor.matmul(
                                ps[:, jj * 128:(jj + 1) * 128],
                                xqT[:, j, c4 * 128:(c4 + 1) * 128],
                                spairs[j],
                                start=True,
                                stop=True,
                                skip_group_check=True,
                            )
                        if half == 0:
                            nc.vector.tensor_copy(oout[:, cc, 0:512], ps)
                        else:
                            nc.scalar.copy(oout[:, cc, 512:1024], ps)
                r0 = blk * R + ch * 128 * grp
                nc.sync.dma_start(
                    o[r0:r0 + 128 * grp, :].rearrange("(c p) m -> p c m", p=128),
                    oout[:, 0:grp, :],
                )

        emit_b(0)
        emit_xq_trans(6)
        emit_b(1)
        emit_xq_trans(7)
        for blk in range(2, NBLK):
            emit_b(blk)

    nc.compile()
    return nc


def _get_program(with_bias=False):
    key = ("nc", bool(with_bias))
    if key not in _CACHE:
        _CACHE[key] = _build_program(with_bias)
    return _CACHE[key]


def make_in_maps(x1, x2, Wkv1, Wkv2, g1_w1, g1_b1, g1_w2, g1_b2,
                 g2_w1, g2_b1, g2_w2, g2_b2):
    """Core (s, b): cores 0-3 = (s=0, b), cores 4-7 = (s=1, b)."""
    import ml_dtypes
    BF = ml_dtypes.bfloat16
    identb = np.eye(128, dtype=BF)
    eye64 = np.eye(64, dtype=np.float32)
    ident2 = np.ascontiguousarray(np.concatenate([eye64, eye64], axis=0))

    def bf(a):
        return np.ascontiguousarray(np.asarray(a, np.float32).astype(BF))

    WSCALE = 16.0  # gate weights scaled into the fp8 normal range

    def f8(a):
        return np.ascontiguousarray(
            (np.asarray(a, np.float32) * WSCALE).astype(ml_dtypes.float8_e4m3fn)
        )

    x1b = [bf(x1[b]) for b in range(x1.shape[0])]
    x2b = [bf(x2[b]) for b in range(x2.shape[0])]
    Ws = [
        dict(wkv=bf(Wkv1), w1=f8(g1_w1), b1=np.asarray(g1_b1, np.float32),
             w2=f8(g1_w2), b2=bf(np.asarray(g1_b2, np.float32) * WSCALE)),
        dict(wkv=bf(Wkv2), w1=f8(g2_w1), b1=np.asarray(g2_b1, np.float32),
             w2=f8(g2_w2), b2=bf(np.asarray(g2_b2, np.float32) * WSCALE)),
    ]
    in_maps = []
    for core in range(8):
        s, b = core // 4, core % 4
        m = dict(Ws[s])
        m["xp"] = x1b[b] if s == 0 else x2b[b]
        m["xq"] = x2b[b] if s == 0 else x1b[b]
        m["identb"] = identb
        m["ident2"] = ident2
        in_maps.append(m)
    return in_maps


def kernel(x1, x2, Wkv1, Wkv2, g1_w1, g1_b1, g1_w2, g1_b2,
           g2_w1, g2_b1, g2_w2, g2_b2, _runner=None):
    """Full-input entry point.  Returns (o1, o2), each [4, 4096, 1024] f32."""
    from concourse.bass_utils import run_bass_kernel_spmd

    args = [np.asarray(a, dtype=np.float32) for a in
            (x1, x2, Wkv1, Wkv2, g1_w1, g1_b1, g1_w2, g1_b2,
             g2_w1, g2_b1, g2_w2, g2_b2)]
    with_bias = bool(np.any(args[7]) or np.any(args[11]))  # g1_b2, g2_b2
    nc = _get_program(with_bias)
    in_maps = make_in_maps(*args)
    if _runner is None:
        res = run_bass_kernel_spmd(nc, in_maps, core_ids=list(range(8)))
        results = res.results
    else:
        results = _runner(nc, in_maps)

    B = x1.shape[0]
    o1 = np.empty((B, N, C), dtype=np.float32)
    o2 = np.empty((B, N, C), dtype=np.float32)
    for core in range(8):
        s, b = core // 4, core % 4
        out = np.asarray(results[core]["o"], dtype=np.float32)
        if s == 0:
            o2[b] = out   # core projected x1 -> ctx1 -> o2 = q2 @ ctx1
        else:
            o1[b] = out
    return (o1, o2)
